# revision 27
# baseline (speedup 1.0000x reference)
"""KNN (K=9, 3 classes) Trainium2 Bass kernel, v3.

Strategy (train-sharded, fold-compressed top-k):
  - X_train split over 8 NeuronCores along N_train (12500 points each,
    zero-padded to 12800 = 25*512); every core scores all 2048 queries
    against its shard:  s[q, n] = 2*q.t_n - ||t_n||^2  (monotone -d2).
  - Per 128-query chunk, 25 tiles of 512 train points; PSUM allocated as
    12 bank-pairs [128,1024] + 1 single [128,512].  Per tile the PE runs
    a rank-3 bf16 matmul (-||t||^2 as a 3-term bf16 split) then a
    float32r matmul (1 cycle/row; ~0.04 abs err) accumulating in fp32.
  - Top-k never materializes [128,12500] in SBUF.  Instead elementwise
    running max over tiles ("fold") onto slot = n mod 512:
      * ScalarE stages most pairs PSUM->SBUF as fp16 (ACTIVATE 1024-wide),
      * DVE folds the staged fp16 pairs at 2 elem/cycle (2x_1p),
      * DVE folds 2 pairs directly from PSUM in fp32,
    then reduces 1024->512, merges streams, and runs one MAX8 +
    FIND_INDEX8 over [128,512] per chunk.
  - Host expands each returned slot p to its 25 candidates {p+512j},
    re-scores all 8*8*25=1600 candidates per query (fp32 batched, then
    fp64 on the global top-64), merges, majority-votes.  The expansion
    provably contains the shard's device-score top-8; queries whose
    margins are within EPS_DEV (float32r + fp16 rounding) are recomputed
    exactly with reference-style fp32 arithmetic.
"""
import os
import sys

sys.path.insert(0, "/opt/trn_rl_repo")

import numpy as np
import ml_dtypes

N_CORES = 8
N_TRAIN = 100000
D = 128
N_TEST = 2048
K = 9
NUM_CLASSES = 3
SHARD = N_TRAIN // N_CORES          # 12500
NTILE = 512
N_TILES = 25
SHARD_PAD = NTILE * N_TILES         # 12800
N_PAIRS = 12                        # 12 pairs + 1 single = 25 tiles
CHUNK = 128
N_CHUNKS = int(os.environ.get("KNN_CHUNKS", N_TEST // CHUNK))  # 16
N_STAGED = int(os.environ.get("KNN_SP", 9))    # pairs staged via ScalarE
N_DIRECT = N_PAIRS - N_STAGED                  # pairs folded from PSUM

_FW = N_TEST + SHARD_PAD            # fblob: [2*X_test.T | X_train_shard.T pad]
_BW = SHARD_PAD + CHUNK             # b_s: [-tn f32r 3-split pad | ones]

# max |device folded value - exact| : float32r matmul (~0.1) + fp16
# staging quantization (~0.06 at |s|<=130).  Rescue margin must be >= 2x.
EPS_DEV = float(os.environ.get("KNN_EPS_DEV", 0.35))
PAD_SCORE = -1e30


def _build_program():
    import concourse.bacc as bacc
    import concourse.mybir as mybir
    from concourse.tile import TileContext

    AF = mybir.ActivationFunctionType
    MAX = mybir.AluOpType.max

    nc = bacc.Bacc("TRN2", target_bir_lowering=False, debug=False)
    f_d = nc.dram_tensor("fblob", [128, _FW], mybir.dt.float32r,
                         kind="ExternalInput").ap()
    b_d = nc.dram_tensor("bblob", [128, _BW], mybir.dt.float32r,
                         kind="ExternalInput").ap()
    o_d = nc.dram_tensor("out", [N_CHUNKS * CHUNK, 16], mybir.dt.float32,
                         kind="ExternalOutput").ap()

    with TileContext(nc) as tc:
        with tc.tile_pool(name="const", bufs=1) as cpool, \
             tc.tile_pool(name="acc", bufs=2) as apool, \
             tc.tile_pool(name="stg", bufs=3) as spool, \
             tc.tile_pool(name="pp", bufs=3, space="PSUM") as pp, \
             tc.tile_pool(name="ps1", bufs=2, space="PSUM") as ps1:
            f_s = cpool.tile([128, _FW], mybir.dt.float32r)
            b_s = cpool.tile([128, _BW], mybir.dt.float32r)
            # b blob: [ones [128,128] | tn split [128,12800]]: rows 0-2 of
            # the tn part hold the -tn 3-split, rows 3-127 zero.  The ones
            # stationary is K=128 so the PE never switches tile config.
            # Interleave fine-grained f/b slices in tile order so chunk-0
            # compute starts after ~1.3 MB and stays just behind the DMA.
            nc.sync.dma_start(f_s[:, 0:N_TEST], f_d[:, 0:N_TEST])
            nc.sync.dma_start(b_s[:, 0:CHUNK], b_d[:, 0:CHUNK])
            _tcuts = [0, 1, 2, 4, 6, 9, 12, 16, 20, N_TILES]
            for a, b in zip(_tcuts[:-1], _tcuts[1:]):
                fa, fb = N_TEST + a * NTILE, N_TEST + b * NTILE
                nc.sync.dma_start(f_s[:, fa:fb], f_d[:, fa:fb])
                ba, bb = CHUNK + a * NTILE, CHUNK + b * NTILE
                nc.sync.dma_start(b_s[:, ba:bb], b_d[:, ba:bb])
            q_s = f_s[:, 0:N_TEST]          # [128, 2048]  (2*X_test).T f32r
            x_s = f_s[:, N_TEST:_FW]        # [128, 12800] shard.T f32r
            ones_s = b_s[:, 0:CHUNK]        # [128, 128]   ones
            tnb_s = b_s[:, CHUNK:_BW]       # [128, 12800] -tn f32r 3-split

            def tile_mms(out_ap, m, j):
                sl = slice(j * NTILE, (j + 1) * NTILE)
                nc.tensor.matmul(out_ap, ones_s[:], tnb_s[:, sl],
                                 start=True, stop=False,
                                 skip_group_check=True)
                nc.tensor.matmul(out_ap, q_s[:, m * CHUNK:(m + 1) * CHUNK],
                                 x_s[:, sl], start=False, stop=True,
                                 skip_group_check=True)

            for m in range(N_CHUNKS):
                accH = apool.tile([128, 1024], mybir.dt.float16, tag="accH")
                accA = apool.tile([128, 1024 + 16], mybir.dt.float32,
                                  tag="accA")
                nc.gpsimd.memset(accA[:, 0:1024], -3e38)
                for p in range(N_PAIRS):
                    ps = pp.tile([128, 1024], mybir.dt.float32, tag="pp")
                    tile_mms(ps[:, 0:NTILE], m, 2 * p)
                    tile_mms(ps[:, NTILE:1024], m, 2 * p + 1)
                    if p < N_DIRECT:
                        nc.vector.tensor_tensor(accA[:, 0:1024],
                                                accA[:, 0:1024], ps[:],
                                                op=MAX)
                    elif p == N_DIRECT:
                        nc.scalar.activation(accH[:], ps[:], AF.Identity)
                    else:
                        tmp = spool.tile([128, 1024], mybir.dt.float16,
                                         tag="tmp")
                        nc.scalar.activation(tmp[:], ps[:], AF.Identity)
                        nc.vector.tensor_tensor(accH[:], accH[:], tmp[:],
                                                op=MAX)
                # single tile j=24
                ss = ps1.tile([128, NTILE], mybir.dt.float32, tag="ss")
                tile_mms(ss[:], m, 24)
                stmp = spool.tile([128, NTILE], mybir.dt.float16, tag="stmp")
                nc.scalar.activation(stmp[:], ss[:], AF.Identity)
                nc.vector.tensor_tensor(accH[:, 0:NTILE], accH[:, 0:NTILE],
                                        stmp[:], op=MAX)
                # reduce 1024 -> 512 in both streams, then merge
                nc.vector.tensor_tensor(accH[:, 0:NTILE], accH[:, 0:NTILE],
                                        accH[:, NTILE:1024], op=MAX)
                nc.vector.tensor_tensor(accA[:, 0:NTILE], accA[:, 0:NTILE],
                                        accA[:, NTILE:1024], op=MAX)
                nc.vector.tensor_tensor(accA[:, 0:NTILE], accA[:, 0:NTILE],
                                        accH[:, 0:NTILE], op=MAX)
                nc.vector.max(accA[:, 1024:1032], accA[:, 0:NTILE])
                nc.vector.max_index(
                    accA[:, 1032:1040].bitcast(mybir.dt.uint32),
                    accA[:, 1024:1032], accA[:, 0:NTILE])
                nc.sync.dma_start(o_d[m * CHUNK:(m + 1) * CHUNK, :],
                                  accA[:, 1024:1040])
    nc.compile()
    return nc


def _round_m10(v64):
    """Round to fp32 with only 10 mantissa bits (exact under f32r)."""
    v = v64.astype(np.float32)
    b = v.view(np.uint32).astype(np.uint64)
    lsb = (b >> np.uint64(13)) & np.uint64(1)
    b = (b + np.uint64(0x0FFF) + lsb) & np.uint64(0xFFFFE000)
    return b.astype(np.uint32).view(np.float32)


def _f32r_split3(v64):
    """3-term split, each term exactly representable at 10 mantissa bits."""
    t0 = _round_m10(v64)
    r1 = v64 - t0.astype(np.float64)
    t1 = _round_m10(r1)
    r2 = r1 - t1.astype(np.float64)
    t2 = _round_m10(r2)
    return np.stack([t0, t1, t2])


def _prep_inputs(X_train, X_test):
    XT = np.ascontiguousarray(X_train.T.astype(np.float32))        # [128,100000]
    q2t = np.ascontiguousarray((2.0 * X_test.astype(np.float32)).T)
    tn64 = (X_train.astype(np.float64) ** 2).sum(1)                # [100000]
    in_maps = []
    for c in range(N_CORES):
        sl = slice(c * SHARD, (c + 1) * SHARD)
        xpad = np.zeros((128, SHARD_PAD), np.float32)
        xpad[:, 0:SHARD] = XT[:, sl]
        tnpad = np.full(SHARD_PAD, -PAD_SCORE, np.float64)
        tnpad[0:SHARD] = tn64[sl]
        fblob = np.ascontiguousarray(np.concatenate([q2t, xpad], axis=1))
        bblob = np.zeros((128, _BW), np.float32)
        bblob[:, 0:CHUNK] = 1.0
        bblob[0:3, CHUNK:_BW] = _f32r_split3(-tnpad)
        in_maps.append({"fblob": fblob, "bblob": bblob})
    return in_maps, tn64


def _reference_style_batch(qs32, X_train32, tn32, qn32):
    """Reference-fp32 top-K for a batch of queries -> [F, K] indices."""
    F = qs32.shape[0]
    out = np.empty((F, K), np.int64)
    B = 64
    for a in range(0, F, B):
        b = min(a + B, F)
        d2 = qn32[a:b][:, None] + tn32[None, :] - 2.0 * (qs32[a:b] @ X_train32.T)
        part = np.argpartition(d2, K + 8, axis=1)[:, :K + 8]
        pv = np.take_along_axis(d2, part, axis=1)
        # stable order by (value, index) to replicate top_k tie-breaking
        ordr = np.lexsort((part, pv), axis=1)[:, :K]
        out[a:b] = np.take_along_axis(part, ordr, axis=1)
    return out


def _host_merge(X_train, y_train, X_test, vals, idxs, tn64, diag=None):
    """vals/idxs: [n_cores, nq, 8] device folded top-8 (values, slot pos).

    Slot p expands to shard candidates {p + 512*j : j in 0..24}.
    """
    nq = vals.shape[1]
    n_cores = vals.shape[0]

    slots = idxs.astype(np.int64)                                  # [C,nq,8]
    bad = (slots >= NTILE).any(axis=(0, 2))                        # [nq]
    ss_ = np.sort(slots, axis=2)
    dup = (np.diff(ss_, axis=2) == 0).any(axis=(0, 2))             # [nq]
    slots = np.clip(slots, 0, NTILE - 1)

    j_off = np.arange(N_TILES, dtype=np.int64) * NTILE             # [25]
    loc = slots[..., None] + j_off                                 # [C,nq,8,25]
    valid = loc < SHARD
    gidx = np.where(valid, loc, 0) + (np.arange(n_cores, dtype=np.int64)
                                      [:, None, None, None] * SHARD)
    n_exp = 8 * N_TILES                                            # 200
    gidx = gidx.reshape(n_cores, nq, n_exp).transpose(1, 0, 2)     # [nq,C,200]
    valid = valid.reshape(n_cores, nq, n_exp).transpose(1, 0, 2)
    gidx_f = gidx.reshape(nq, n_cores * n_exp)
    valid_f = valid.reshape(nq, n_cores * n_exp)

    # fp32 re-score of all candidates: s = 2 q.t - ||t||^2
    X32 = X_train.astype(np.float32)
    tn32 = (X32.astype(np.float64) ** 2).sum(1).astype(np.float32)
    q32 = X_test.astype(np.float32)
    s32 = np.empty((nq, n_cores * n_exp), dtype=np.float32)
    QB = 256
    for a in range(0, nq, QB):
        b = min(a + QB, nq)
        Xc = X32[gidx_f[a:b]]                                      # [B,1600,128]
        s32[a:b] = (2.0 * np.matmul(Xc, q32[a:b, :, None])[..., 0]
                    - tn32[gidx_f[a:b]])
    s32[~valid_f] = PAD_SCORE

    # per-core 8th-best (fp32) for the hidden-candidate rescue bound
    s_core = s32.reshape(nq, n_cores, n_exp)
    e8 = -np.partition(-s_core, 7, axis=2)[:, :, 7]                # [nq,C]

    # device folded values vs fp32 slot maxima: EPS_DEV sanity
    slot_max = s_core.reshape(nq, n_cores, 8, N_TILES).max(3)
    dev_err = np.abs(vals.transpose(1, 0, 2).astype(np.float32) - slot_max)
    if diag is not None:
        diag["max_dev_err"] = float(dev_err.max())
        diag["p99_dev_err"] = float(np.quantile(dev_err, 0.99))
    flag_eps = (dev_err > 0.5 * EPS_DEV).any(axis=(1, 2))

    # global top-64 by fp32, re-scored in fp64 for exact ordering
    TOP = 64
    top_i = np.argpartition(-s32, TOP - 1, axis=1)[:, :TOP]        # [nq,64]
    cand = np.take_along_axis(gidx_f, top_i, axis=1)               # [nq,64]
    Xc64 = X_train[cand].astype(np.float64)                        # [nq,64,128]
    s64 = (2.0 * np.matmul(Xc64,
                           X_test.astype(np.float64)[:, :, None])[..., 0]
           - tn64[cand])
    cand_valid = np.take_along_axis(valid_f, top_i, axis=1)
    s64[~cand_valid] = PAD_SCORE
    order = np.argsort(-s64, axis=1, kind="stable")
    s_sorted = np.take_along_axis(s64, order, axis=1)
    top9 = np.take_along_axis(cand, order[:, :K], axis=1)          # [nq,9]
    v9 = s_sorted[:, K - 1]
    v10 = s_sorted[:, K]

    flag_hidden = (e8.astype(np.float64) + EPS_DEV >= v9[:, None]).any(1)
    top9_core = top9 // SHARD
    counts_core = np.zeros((nq, n_cores), dtype=np.int64)
    for c in range(n_cores):
        counts_core[:, c] = (top9_core == c).sum(1)
    flag_dom = (counts_core >= 8).any(1)
    flag_tie = (v9 - v10) < 1e-4
    # duplicate candidates in the merged top-9 (shouldn't happen; safety)
    t9s = np.sort(top9, axis=1)
    dup9 = (np.diff(t9s, axis=1) == 0).any(1)

    flagged = np.where(bad | dup | flag_hidden | flag_dom | flag_tie
                       | flag_eps | dup9)[0]
    if len(flagged):
        qn32 = (q32 * q32).sum(1)
        top9[flagged] = _reference_style_batch(q32[flagged], X32, tn32,
                                               qn32[flagged])

    labels = y_train[top9]                                         # [nq,9]
    counts = (labels[:, :, None] ==
              np.arange(NUM_CLASSES, dtype=labels.dtype)[None, None, :]).sum(1)
    preds = counts.argmax(1).astype(np.int32)
    return preds, len(flagged)


_cached = {}


def run_device(X_train, X_test, trace=False):
    from concourse.bass_utils import run_bass_kernel_spmd

    in_maps, tn64 = _prep_inputs(X_train, X_test)
    if "nc" not in _cached:
        _cached["nc"] = _build_program()
    nc = _cached["nc"]
    res = run_bass_kernel_spmd(nc, in_maps, core_ids=list(range(N_CORES)),
                               trace=trace)
    vals = np.stack([res.results[c]["out"][:, 0:8] for c in range(N_CORES)])
    idxs = np.stack([res.results[c]["out"][:, 8:16].view(np.uint32)
                     for c in range(N_CORES)])
    return vals, idxs, tn64, res


def kernel(X_train, y_train, X_test):
    X_train = np.asarray(X_train)
    y_train = np.asarray(y_train)
    X_test = np.asarray(X_test)
    vals, idxs, tn64, _ = run_device(X_train, X_test, trace=False)
    nq = vals.shape[1]
    preds, _n_flagged = _host_merge(X_train, y_train, X_test[:nq], vals, idxs,
                                    tn64)
    return preds


# revision 28
# speedup vs baseline: 1.1771x; 1.1771x over previous
"""KNN (K=9, 3 classes) Trainium2 Bass kernel, v3.

Strategy (train-sharded, fold-compressed top-k):
  - X_train split over 8 NeuronCores along N_train (12500 points each,
    zero-padded to 12800 = 25*512); every core scores all 2048 queries
    against its shard:  s[q, n] = 2*q.t_n - ||t_n||^2  (monotone -d2).
  - Per 128-query chunk, 25 tiles of 512 train points; PSUM allocated as
    12 bank-pairs [128,1024] + 1 single [128,512].  Per tile the PE runs
    a rank-3 bf16 matmul (-||t||^2 as a 3-term bf16 split) then a
    float32r matmul (1 cycle/row; ~0.04 abs err) accumulating in fp32.
  - Top-k never materializes [128,12500] in SBUF.  Instead elementwise
    running max over tiles ("fold") onto slot = n mod 512:
      * ScalarE stages most pairs PSUM->SBUF as fp16 (ACTIVATE 1024-wide),
      * DVE folds the staged fp16 pairs at 2 elem/cycle (2x_1p),
      * DVE folds 2 pairs directly from PSUM in fp32,
    then reduces 1024->512, merges streams, and runs one MAX8 +
    FIND_INDEX8 over [128,512] per chunk.
  - Host expands each returned slot p to its 25 candidates {p+512j},
    re-scores all 8*8*25=1600 candidates per query (fp32 batched, then
    fp64 on the global top-64), merges, majority-votes.  The expansion
    provably contains the shard's device-score top-8; queries whose
    margins are within EPS_DEV (float32r + fp16 rounding) are recomputed
    exactly with reference-style fp32 arithmetic.
"""
import os
import sys

sys.path.insert(0, "/opt/trn_rl_repo")

import numpy as np
import ml_dtypes

N_CORES = 8
N_TRAIN = 100000
D = 128
N_TEST = 2048
K = 9
NUM_CLASSES = 3
SHARD = N_TRAIN // N_CORES          # 12500
NTILE = 512
N_TILES = 25
SHARD_PAD = NTILE * N_TILES         # 12800
N_PAIRS = 12                        # 12 pairs + 1 single = 25 tiles
CHUNK = 128
N_CHUNKS = int(os.environ.get("KNN_CHUNKS", N_TEST // CHUNK))  # 16
N_STAGED = int(os.environ.get("KNN_SP", 9))    # pairs staged via ScalarE
N_DIRECT = N_PAIRS - N_STAGED                  # pairs folded from PSUM

_FW = N_TEST + SHARD_PAD            # fblob: [2*X_test.T | X_train_shard.T pad]
_BW = SHARD_PAD + CHUNK             # b_s: [-tn f32r 3-split pad | ones]

# max |device folded value - exact| : float32r matmul (~0.1) + fp16
# staging quantization (~0.06 at |s|<=130).  Rescue margin must be >= 2x.
EPS_DEV = float(os.environ.get("KNN_EPS_DEV", 0.35))
PAD_SCORE = -1e30


def _build_program():
    import concourse.bacc as bacc
    import concourse.mybir as mybir
    from concourse.tile import TileContext

    AF = mybir.ActivationFunctionType
    MAX = mybir.AluOpType.max

    nc = bacc.Bacc("TRN2", target_bir_lowering=False, debug=False)
    f_d = nc.dram_tensor("fblob", [128, _FW], mybir.dt.float32r,
                         kind="ExternalInput").ap()
    b_d = nc.dram_tensor("bblob", [128, _BW], mybir.dt.float32r,
                         kind="ExternalInput").ap()
    o_d = nc.dram_tensor("out", [N_CHUNKS * CHUNK, 16], mybir.dt.float32,
                         kind="ExternalOutput").ap()

    with TileContext(nc) as tc:
        with tc.tile_pool(name="const", bufs=1) as cpool, \
             tc.tile_pool(name="acc", bufs=2) as apool, \
             tc.tile_pool(name="stg", bufs=3) as spool, \
             tc.tile_pool(name="pp", bufs=3, space="PSUM") as pp, \
             tc.tile_pool(name="ps1", bufs=2, space="PSUM") as ps1:
            f_s = cpool.tile([128, _FW], mybir.dt.float32r)
            b_s = cpool.tile([128, _BW], mybir.dt.float32r)
            # b blob: [ones [128,128] | tn split [128,12800]]: rows 0-2 of
            # the tn part hold the -tn 3-split, rows 3-127 zero.  The ones
            # stationary is K=128 so the PE never switches tile config.
            # Coarse interleaved slices: finer slicing makes the DMA trickle
            # through the whole run and contend with compute.
            _bcuts = [0, CHUNK + 7 * NTILE, CHUNK + 13 * NTILE,
                      CHUNK + 19 * NTILE, _BW]
            for a, b in zip(_bcuts[:-1], _bcuts[1:]):
                nc.sync.dma_start(b_s[:, a:b], b_d[:, a:b])
            _cuts = [0, N_TEST + 7 * NTILE, N_TEST + 13 * NTILE,
                     N_TEST + 19 * NTILE, _FW]
            for a, b in zip(_cuts[:-1], _cuts[1:]):
                nc.sync.dma_start(f_s[:, a:b], f_d[:, a:b])
            q_s = f_s[:, 0:N_TEST]          # [128, 2048]  (2*X_test).T f32r
            x_s = f_s[:, N_TEST:_FW]        # [128, 12800] shard.T f32r
            ones_s = b_s[:, 0:CHUNK]        # [128, 128]   ones
            tnb_s = b_s[:, CHUNK:_BW]       # [128, 12800] -tn f32r 3-split

            def tile_mms(out_ap, m, j):
                sl = slice(j * NTILE, (j + 1) * NTILE)
                nc.tensor.matmul(out_ap, ones_s[:], tnb_s[:, sl],
                                 start=True, stop=False,
                                 skip_group_check=True)
                nc.tensor.matmul(out_ap, q_s[:, m * CHUNK:(m + 1) * CHUNK],
                                 x_s[:, sl], start=False, stop=True,
                                 skip_group_check=True)

            for m in range(N_CHUNKS):
                accH = apool.tile([128, 1024], mybir.dt.float16, tag="accH")
                accA = apool.tile([128, 1024 + 16], mybir.dt.float32,
                                  tag="accA")
                nc.gpsimd.memset(accA[:, 0:1024], -3e38)
                for p in range(N_PAIRS):
                    ps = pp.tile([128, 1024], mybir.dt.float32, tag="pp")
                    tile_mms(ps[:, 0:NTILE], m, 2 * p)
                    tile_mms(ps[:, NTILE:1024], m, 2 * p + 1)
                    if p < N_DIRECT:
                        nc.vector.tensor_tensor(accA[:, 0:1024],
                                                accA[:, 0:1024], ps[:],
                                                op=MAX)
                    elif p == N_DIRECT:
                        nc.scalar.activation(accH[:], ps[:], AF.Identity)
                    else:
                        tmp = spool.tile([128, 1024], mybir.dt.float16,
                                         tag="tmp")
                        nc.scalar.activation(tmp[:], ps[:], AF.Identity)
                        nc.vector.tensor_tensor(accH[:], accH[:], tmp[:],
                                                op=MAX)
                # single tile j=24
                ss = ps1.tile([128, NTILE], mybir.dt.float32, tag="ss")
                tile_mms(ss[:], m, 24)
                stmp = spool.tile([128, NTILE], mybir.dt.float16, tag="stmp")
                nc.scalar.activation(stmp[:], ss[:], AF.Identity)
                nc.vector.tensor_tensor(accH[:, 0:NTILE], accH[:, 0:NTILE],
                                        stmp[:], op=MAX)
                # reduce 1024 -> 512 in both streams, then merge
                nc.vector.tensor_tensor(accH[:, 0:NTILE], accH[:, 0:NTILE],
                                        accH[:, NTILE:1024], op=MAX)
                nc.vector.tensor_tensor(accA[:, 0:NTILE], accA[:, 0:NTILE],
                                        accA[:, NTILE:1024], op=MAX)
                nc.vector.tensor_tensor(accA[:, 0:NTILE], accA[:, 0:NTILE],
                                        accH[:, 0:NTILE], op=MAX)
                nc.vector.max(accA[:, 1024:1032], accA[:, 0:NTILE])
                nc.vector.max_index(
                    accA[:, 1032:1040].bitcast(mybir.dt.uint32),
                    accA[:, 1024:1032], accA[:, 0:NTILE])
                nc.sync.dma_start(o_d[m * CHUNK:(m + 1) * CHUNK, :],
                                  accA[:, 1024:1040])
    nc.compile()
    return nc


def _round_m10(v64):
    """Round to fp32 with only 10 mantissa bits (exact under f32r)."""
    v = v64.astype(np.float32)
    b = v.view(np.uint32).astype(np.uint64)
    lsb = (b >> np.uint64(13)) & np.uint64(1)
    b = (b + np.uint64(0x0FFF) + lsb) & np.uint64(0xFFFFE000)
    return b.astype(np.uint32).view(np.float32)


def _f32r_split3(v64):
    """3-term split, each term exactly representable at 10 mantissa bits."""
    t0 = _round_m10(v64)
    r1 = v64 - t0.astype(np.float64)
    t1 = _round_m10(r1)
    r2 = r1 - t1.astype(np.float64)
    t2 = _round_m10(r2)
    return np.stack([t0, t1, t2])


def _prep_inputs(X_train, X_test):
    XT = np.ascontiguousarray(X_train.T.astype(np.float32))        # [128,100000]
    q2t = np.ascontiguousarray((2.0 * X_test.astype(np.float32)).T)
    tn64 = (X_train.astype(np.float64) ** 2).sum(1)                # [100000]
    in_maps = []
    for c in range(N_CORES):
        sl = slice(c * SHARD, (c + 1) * SHARD)
        xpad = np.zeros((128, SHARD_PAD), np.float32)
        xpad[:, 0:SHARD] = XT[:, sl]
        tnpad = np.full(SHARD_PAD, -PAD_SCORE, np.float64)
        tnpad[0:SHARD] = tn64[sl]
        fblob = np.ascontiguousarray(np.concatenate([q2t, xpad], axis=1))
        bblob = np.zeros((128, _BW), np.float32)
        bblob[:, 0:CHUNK] = 1.0
        bblob[0:3, CHUNK:_BW] = _f32r_split3(-tnpad)
        in_maps.append({"fblob": fblob, "bblob": bblob})
    return in_maps, tn64


def _reference_style_batch(qs32, X_train32, tn32, qn32):
    """Reference-fp32 top-K for a batch of queries -> [F, K] indices."""
    F = qs32.shape[0]
    out = np.empty((F, K), np.int64)
    B = 64
    for a in range(0, F, B):
        b = min(a + B, F)
        d2 = qn32[a:b][:, None] + tn32[None, :] - 2.0 * (qs32[a:b] @ X_train32.T)
        part = np.argpartition(d2, K + 8, axis=1)[:, :K + 8]
        pv = np.take_along_axis(d2, part, axis=1)
        # stable order by (value, index) to replicate top_k tie-breaking
        ordr = np.lexsort((part, pv), axis=1)[:, :K]
        out[a:b] = np.take_along_axis(part, ordr, axis=1)
    return out


def _host_merge(X_train, y_train, X_test, vals, idxs, tn64, diag=None):
    """vals/idxs: [n_cores, nq, 8] device folded top-8 (values, slot pos).

    Slot p expands to shard candidates {p + 512*j : j in 0..24}.
    """
    nq = vals.shape[1]
    n_cores = vals.shape[0]

    slots = idxs.astype(np.int64)                                  # [C,nq,8]
    bad = (slots >= NTILE).any(axis=(0, 2))                        # [nq]
    ss_ = np.sort(slots, axis=2)
    dup = (np.diff(ss_, axis=2) == 0).any(axis=(0, 2))             # [nq]
    slots = np.clip(slots, 0, NTILE - 1)

    j_off = np.arange(N_TILES, dtype=np.int64) * NTILE             # [25]
    loc = slots[..., None] + j_off                                 # [C,nq,8,25]
    valid = loc < SHARD
    gidx = np.where(valid, loc, 0) + (np.arange(n_cores, dtype=np.int64)
                                      [:, None, None, None] * SHARD)
    n_exp = 8 * N_TILES                                            # 200
    gidx = gidx.reshape(n_cores, nq, n_exp).transpose(1, 0, 2)     # [nq,C,200]
    valid = valid.reshape(n_cores, nq, n_exp).transpose(1, 0, 2)
    gidx_f = gidx.reshape(nq, n_cores * n_exp)
    valid_f = valid.reshape(nq, n_cores * n_exp)

    # fp32 re-score of all candidates: s = 2 q.t - ||t||^2
    X32 = X_train.astype(np.float32)
    tn32 = (X32.astype(np.float64) ** 2).sum(1).astype(np.float32)
    q32 = X_test.astype(np.float32)
    s32 = np.empty((nq, n_cores * n_exp), dtype=np.float32)
    QB = 256
    for a in range(0, nq, QB):
        b = min(a + QB, nq)
        Xc = X32[gidx_f[a:b]]                                      # [B,1600,128]
        s32[a:b] = (2.0 * np.matmul(Xc, q32[a:b, :, None])[..., 0]
                    - tn32[gidx_f[a:b]])
    s32[~valid_f] = PAD_SCORE

    # per-core 8th-best (fp32) for the hidden-candidate rescue bound
    s_core = s32.reshape(nq, n_cores, n_exp)
    e8 = -np.partition(-s_core, 7, axis=2)[:, :, 7]                # [nq,C]

    # device folded values vs fp32 slot maxima: EPS_DEV sanity
    slot_max = s_core.reshape(nq, n_cores, 8, N_TILES).max(3)
    dev_err = np.abs(vals.transpose(1, 0, 2).astype(np.float32) - slot_max)
    if diag is not None:
        diag["max_dev_err"] = float(dev_err.max())
        diag["p99_dev_err"] = float(np.quantile(dev_err, 0.99))
    flag_eps = (dev_err > 0.5 * EPS_DEV).any(axis=(1, 2))

    # global top-64 by fp32, re-scored in fp64 for exact ordering
    TOP = 64
    top_i = np.argpartition(-s32, TOP - 1, axis=1)[:, :TOP]        # [nq,64]
    cand = np.take_along_axis(gidx_f, top_i, axis=1)               # [nq,64]
    Xc64 = X_train[cand].astype(np.float64)                        # [nq,64,128]
    s64 = (2.0 * np.matmul(Xc64,
                           X_test.astype(np.float64)[:, :, None])[..., 0]
           - tn64[cand])
    cand_valid = np.take_along_axis(valid_f, top_i, axis=1)
    s64[~cand_valid] = PAD_SCORE
    order = np.argsort(-s64, axis=1, kind="stable")
    s_sorted = np.take_along_axis(s64, order, axis=1)
    top9 = np.take_along_axis(cand, order[:, :K], axis=1)          # [nq,9]
    v9 = s_sorted[:, K - 1]
    v10 = s_sorted[:, K]

    flag_hidden = (e8.astype(np.float64) + EPS_DEV >= v9[:, None]).any(1)
    top9_core = top9 // SHARD
    counts_core = np.zeros((nq, n_cores), dtype=np.int64)
    for c in range(n_cores):
        counts_core[:, c] = (top9_core == c).sum(1)
    flag_dom = (counts_core >= 8).any(1)
    flag_tie = (v9 - v10) < 1e-4
    # duplicate candidates in the merged top-9 (shouldn't happen; safety)
    t9s = np.sort(top9, axis=1)
    dup9 = (np.diff(t9s, axis=1) == 0).any(1)

    flagged = np.where(bad | dup | flag_hidden | flag_dom | flag_tie
                       | flag_eps | dup9)[0]
    if len(flagged):
        qn32 = (q32 * q32).sum(1)
        top9[flagged] = _reference_style_batch(q32[flagged], X32, tn32,
                                               qn32[flagged])

    labels = y_train[top9]                                         # [nq,9]
    counts = (labels[:, :, None] ==
              np.arange(NUM_CLASSES, dtype=labels.dtype)[None, None, :]).sum(1)
    preds = counts.argmax(1).astype(np.int32)
    return preds, len(flagged)


_cached = {}


def run_device(X_train, X_test, trace=False):
    from concourse.bass_utils import run_bass_kernel_spmd

    in_maps, tn64 = _prep_inputs(X_train, X_test)
    if "nc" not in _cached:
        _cached["nc"] = _build_program()
    nc = _cached["nc"]
    res = run_bass_kernel_spmd(nc, in_maps, core_ids=list(range(N_CORES)),
                               trace=trace)
    vals = np.stack([res.results[c]["out"][:, 0:8] for c in range(N_CORES)])
    idxs = np.stack([res.results[c]["out"][:, 8:16].view(np.uint32)
                     for c in range(N_CORES)])
    return vals, idxs, tn64, res


def kernel(X_train, y_train, X_test):
    X_train = np.asarray(X_train)
    y_train = np.asarray(y_train)
    X_test = np.asarray(X_test)
    vals, idxs, tn64, _ = run_device(X_train, X_test, trace=False)
    nq = vals.shape[1]
    preds, _n_flagged = _host_merge(X_train, y_train, X_test[:nq], vals, idxs,
                                    tn64)
    return preds


# revision 29
# speedup vs baseline: 1.2637x; 1.0736x over previous
"""KNN (K=9, 3 classes) Trainium2 Bass kernel, v6 (transposed layout).

Train points live on PSUM partitions, queries on the free dim, so the
-||t||^2 term is a per-partition bias applied during the PSUM exit
(ScalarE ACTIVATE bias / DVE scalar_tensor_tensor scalar) -- the PE
runs a pure float32r matmul stream with no tn matmuls at all:

  - X_train split over 8 cores along N_train (12500 -> 100 x-tiles of
    128 points, zero-padded).  Per x-tile t the PE computes
    [128 points, 2048 queries] = x_t.T @ (2*X_test.T) as 4 matmuls of
    512 queries into 2 PSUM bank-pairs.
  - x-tiles fold (elementwise running max over tiles) into 4 group
    accumulators, g = t mod 4:
      groups 0-2: ScalarE stages PSUM->SBUF fp16 (ACTIVATE Identity
        with bias = -tn per partition), DVE folds fp16 at 2x;
      group 3: DVE scalar_tensor_tensor (ps + (-tn)) max acc, fp32,
        straight from PSUM.
  - Finals (once): fp16 accs convert to fp32, PE transposes each
    [128 points,128 queries] block (identity-permutation matmul) so
    queries land on partitions, ScalarE exits [128,512] slot arrays,
    one MAX8 + FIND_INDEX8 per 128-query chunk.
  - Slot p = 128*g + r expands to candidates {(g+4j)*128 + r, j<25};
    the host re-scores all 8*8*25 candidates per (core,query), merges
    exactly, majority-votes; margin cases are recomputed exactly.
"""
import os
import sys

sys.path.insert(0, "/opt/trn_rl_repo")

import numpy as np

N_CORES = 8
N_TRAIN = 100000
D = 128
N_TEST = 2048
K = 9
NUM_CLASSES = 3
SHARD = N_TRAIN // N_CORES          # 12500
XTILE = 128
N_XT = 100                          # x-tiles per core (padded)
SHARD_PAD = XTILE * N_XT            # 12800
N_GROUPS = 4
N_JT = N_XT // N_GROUPS             # 25 tiles per group
CHUNK = 128
N_CHUNKS = N_TEST // CHUNK          # 16
NSLOT = N_GROUPS * XTILE            # 512 slot space

_FW = N_TEST + SHARD_PAD

EPS_DEV = float(os.environ.get("KNN_EPS_DEV", 0.35))
PAD_SCORE = -1e30


def _build_program():
    import concourse.bacc as bacc
    import concourse.mybir as mybir
    from concourse.tile import TileContext

    AF = mybir.ActivationFunctionType
    MAX = mybir.AluOpType.max
    ADD = mybir.AluOpType.add

    nc = bacc.Bacc("TRN2", target_bir_lowering=False, debug=False)
    f_d = nc.dram_tensor("fblob", [128, _FW], mybir.dt.float32r,
                         kind="ExternalInput").ap()
    t_d = nc.dram_tensor("tnt", [128, N_XT], mybir.dt.float32,
                         kind="ExternalInput").ap()
    i_d = nc.dram_tensor("iden", [128, 128], mybir.dt.float32,
                         kind="ExternalInput").ap()
    o_d = nc.dram_tensor("out", [N_CHUNKS * CHUNK, 16], mybir.dt.float32,
                         kind="ExternalOutput").ap()

    with TileContext(nc) as tc:
        with tc.tile_pool(name="const", bufs=1) as cpool, \
             tc.tile_pool(name="stg", bufs=6) as spool, \
             tc.tile_pool(name="fin", bufs=2) as fpool, \
             tc.tile_pool(name="pp", bufs=2, space="PSUM") as pp:
            f_s = cpool.tile([128, _FW], mybir.dt.float32r)
            tnt = cpool.tile([128, N_XT], mybir.dt.float32)
            iden = cpool.tile([128, 128], mybir.dt.float32)
            nc.sync.dma_start(tnt[:], t_d[:])
            nc.sync.dma_start(iden[:], i_d[:])
            _cuts = [0, N_TEST + 25 * XTILE, N_TEST + 50 * XTILE,
                     N_TEST + 75 * XTILE, _FW]
            for a, b in zip(_cuts[:-1], _cuts[1:]):
                nc.sync.dma_start(f_s[:, a:b], f_d[:, a:b])
            q_s = f_s[:, 0:N_TEST]          # [128, 2048] (2*X_test).T f32r
            x_s = f_s[:, N_TEST:_FW]        # [128, 12800] shard.T f32r

            accH = [cpool.tile([128, 2048], mybir.dt.float16,
                               tag=f"accH{g}", name=f"accH{g}")
                    for g in range(3)]
            accA = cpool.tile([128, 2048], mybir.dt.float32, tag="accA")
            accF = [cpool.tile([128, 2048], mybir.dt.float32,
                               tag=f"accF{g}", name=f"accF{g}")
                    for g in range(3)]
            nc.gpsimd.memset(accA[:], -3e38)

            for t in range(N_XT):
                g = t % N_GROUPS
                xt = x_s[:, t * XTILE:(t + 1) * XTILE]
                bias = tnt[:, t:t + 1]
                ps = pp.tile([128, 2048], mybir.dt.float32, tag="pp")
                for h in range(4):
                    nc.tensor.matmul(ps[:, h * 512:(h + 1) * 512], xt,
                                     q_s[:, h * 512:(h + 1) * 512],
                                     start=True, stop=True,
                                     skip_group_check=True)
                if g == 3:
                    nc.vector.scalar_tensor_tensor(
                        accA[:], ps[:], bias, accA[:], op0=ADD, op1=MAX)
                elif t == g:
                    nc.scalar.activation(accH[g][:], ps[:], AF.Identity,
                                         bias=bias)
                else:
                    tmp = spool.tile([128, 2048], mybir.dt.float16,
                                     tag="tmp")
                    nc.scalar.activation(tmp[:], ps[:], AF.Identity,
                                         bias=bias)
                    nc.vector.tensor_tensor(accH[g][:], accH[g][:], tmp[:],
                                            op=MAX)
            # finals
            for g in range(3):
                nc.scalar.activation(accF[g][:], accH[g][:], AF.Identity)
            srcs = accF + [accA]
            for c in range(N_CHUNKS):
                psT = pp.tile([128, 2048], mybir.dt.float32, tag="pp")
                for g in range(N_GROUPS):
                    nc.tensor.matmul(
                        psT[:, g * 128:(g + 1) * 128],
                        srcs[g][:, c * CHUNK:(c + 1) * CHUNK], iden[:],
                        start=True, stop=True, is_transpose=True,
                        skip_group_check=True)
                fin = fpool.tile([128, NSLOT + 16], mybir.dt.float32,
                                 tag="fin")
                nc.scalar.copy(fin[:, 0:NSLOT], psT[:, 0:NSLOT])
                nc.vector.max(fin[:, NSLOT:NSLOT + 8], fin[:, 0:NSLOT])
                nc.vector.max_index(
                    fin[:, NSLOT + 8:NSLOT + 16].bitcast(mybir.dt.uint32),
                    fin[:, NSLOT:NSLOT + 8], fin[:, 0:NSLOT])
                nc.sync.dma_start(o_d[c * CHUNK:(c + 1) * CHUNK, :],
                                  fin[:, NSLOT:NSLOT + 16])
    nc.compile()
    return nc


def _prep_inputs(X_train, X_test):
    XT = np.ascontiguousarray(X_train.T.astype(np.float32))        # [128,100000]
    q2t = np.ascontiguousarray((2.0 * X_test.astype(np.float32)).T)
    tn64 = (X_train.astype(np.float64) ** 2).sum(1)                # [100000]
    iden = np.eye(128, dtype=np.float32)
    in_maps = []
    for c in range(N_CORES):
        sl = slice(c * SHARD, (c + 1) * SHARD)
        xpad = np.zeros((128, SHARD_PAD), np.float32)
        xpad[:, 0:SHARD] = XT[:, sl]
        tnpad = np.full(SHARD_PAD, PAD_SCORE, np.float64)
        tnpad[0:SHARD] = -tn64[sl]
        fblob = np.ascontiguousarray(np.concatenate([q2t, xpad], axis=1))
        tnt = np.ascontiguousarray(
            tnpad.reshape(N_XT, XTILE).T.astype(np.float32))       # [128,100]
        in_maps.append({"fblob": fblob, "tnt": tnt, "iden": iden})
    return in_maps, tn64


def _reference_style_batch(qs32, X_train32, tn32, qn32):
    """Reference-fp32 top-K for a batch of queries -> [F, K] indices."""
    F = qs32.shape[0]
    out = np.empty((F, K), np.int64)
    B = 64
    for a in range(0, F, B):
        b = min(a + B, F)
        d2 = qn32[a:b][:, None] + tn32[None, :] - 2.0 * (qs32[a:b] @ X_train32.T)
        part = np.argpartition(d2, K + 8, axis=1)[:, :K + 8]
        pv = np.take_along_axis(d2, part, axis=1)
        ordr = np.lexsort((part, pv), axis=1)[:, :K]
        out[a:b] = np.take_along_axis(part, ordr, axis=1)
    return out


def _host_merge(X_train, y_train, X_test, vals, idxs, tn64, diag=None):
    """vals/idxs: [n_cores, nq, 8] device folded top-8 (values, slot).

    Slot p = 128*g + r expands to {(g + 4*j)*128 + r : j in 0..24}.
    """
    nq = vals.shape[1]
    n_cores = vals.shape[0]

    slots = idxs.astype(np.int64)                                  # [C,nq,8]
    bad = (slots >= NSLOT).any(axis=(0, 2))
    ss_ = np.sort(slots, axis=2)
    dup = (np.diff(ss_, axis=2) == 0).any(axis=(0, 2))
    slots = np.clip(slots, 0, NSLOT - 1)

    gg = slots // XTILE                                            # [C,nq,8]
    rr = slots % XTILE
    jj = np.arange(N_JT, dtype=np.int64)                           # [25]
    loc = (gg[..., None] + N_GROUPS * jj) * XTILE + rr[..., None]  # [C,nq,8,25]
    valid = loc < SHARD
    gidx = np.where(valid, loc, 0) + (np.arange(n_cores, dtype=np.int64)
                                      [:, None, None, None] * SHARD)
    n_exp = 8 * N_JT                                               # 200
    gidx = gidx.reshape(n_cores, nq, n_exp).transpose(1, 0, 2)
    valid = valid.reshape(n_cores, nq, n_exp).transpose(1, 0, 2)
    gidx_f = gidx.reshape(nq, n_cores * n_exp)
    valid_f = valid.reshape(nq, n_cores * n_exp)

    X32 = X_train.astype(np.float32)
    tn32 = (X32.astype(np.float64) ** 2).sum(1).astype(np.float32)
    q32 = X_test.astype(np.float32)
    s32 = np.empty((nq, n_cores * n_exp), dtype=np.float32)
    QB = 256
    for a in range(0, nq, QB):
        b = min(a + QB, nq)
        Xc = X32[gidx_f[a:b]]
        s32[a:b] = (2.0 * np.matmul(Xc, q32[a:b, :, None])[..., 0]
                    - tn32[gidx_f[a:b]])
    s32[~valid_f] = PAD_SCORE

    s_core = s32.reshape(nq, n_cores, n_exp)
    e8 = -np.partition(-s_core, 7, axis=2)[:, :, 7]

    slot_max = s_core.reshape(nq, n_cores, 8, N_JT).max(3)
    dev_err = np.abs(vals.transpose(1, 0, 2).astype(np.float32) - slot_max)
    if diag is not None:
        diag["max_dev_err"] = float(dev_err.max())
        diag["p99_dev_err"] = float(np.quantile(dev_err, 0.99))
    flag_eps = (dev_err > 0.5 * EPS_DEV).any(axis=(1, 2))

    TOP = 64
    top_i = np.argpartition(-s32, TOP - 1, axis=1)[:, :TOP]
    cand = np.take_along_axis(gidx_f, top_i, axis=1)
    Xc64 = X_train[cand].astype(np.float64)
    s64 = (2.0 * np.matmul(Xc64,
                           X_test.astype(np.float64)[:, :, None])[..., 0]
           - tn64[cand])
    cand_valid = np.take_along_axis(valid_f, top_i, axis=1)
    s64[~cand_valid] = PAD_SCORE
    order = np.argsort(-s64, axis=1, kind="stable")
    s_sorted = np.take_along_axis(s64, order, axis=1)
    top9 = np.take_along_axis(cand, order[:, :K], axis=1)
    v9 = s_sorted[:, K - 1]
    v10 = s_sorted[:, K]

    flag_hidden = (e8.astype(np.float64) + EPS_DEV >= v9[:, None]).any(1)
    top9_core = top9 // SHARD
    counts_core = np.zeros((nq, n_cores), dtype=np.int64)
    for c in range(n_cores):
        counts_core[:, c] = (top9_core == c).sum(1)
    flag_dom = (counts_core >= 8).any(1)
    flag_tie = (v9 - v10) < 1e-4
    t9s = np.sort(top9, axis=1)
    dup9 = (np.diff(t9s, axis=1) == 0).any(1)

    flagged = np.where(bad | dup | flag_hidden | flag_dom | flag_tie
                       | flag_eps | dup9)[0]
    if len(flagged):
        qn32 = (q32 * q32).sum(1)
        top9[flagged] = _reference_style_batch(q32[flagged], X32, tn32,
                                               qn32[flagged])

    labels = y_train[top9]
    counts = (labels[:, :, None] ==
              np.arange(NUM_CLASSES, dtype=labels.dtype)[None, None, :]).sum(1)
    preds = counts.argmax(1).astype(np.int32)
    return preds, len(flagged)


_cached = {}


def run_device(X_train, X_test, trace=False):
    from concourse.bass_utils import run_bass_kernel_spmd

    in_maps, tn64 = _prep_inputs(X_train, X_test)
    if "nc" not in _cached:
        _cached["nc"] = _build_program()
    nc = _cached["nc"]
    res = run_bass_kernel_spmd(nc, in_maps, core_ids=list(range(N_CORES)),
                               trace=trace)
    vals = np.stack([res.results[c]["out"][:, 0:8] for c in range(N_CORES)])
    idxs = np.stack([res.results[c]["out"][:, 8:16].view(np.uint32)
                     for c in range(N_CORES)])
    return vals, idxs, tn64, res


def kernel(X_train, y_train, X_test):
    X_train = np.asarray(X_train)
    y_train = np.asarray(y_train)
    X_test = np.asarray(X_test)
    vals, idxs, tn64, _ = run_device(X_train, X_test, trace=False)
    nq = vals.shape[1]
    preds, _n_flagged = _host_merge(X_train, y_train, X_test[:nq], vals, idxs,
                                    tn64)
    return preds


# revision 30
# speedup vs baseline: 1.2851x; 1.0169x over previous
"""KNN (K=9, 3 classes) Trainium2 Bass kernel, v6 (transposed layout).

Train points live on PSUM partitions, queries on the free dim, so the
-||t||^2 term is a per-partition bias applied during the PSUM exit
(ScalarE ACTIVATE bias / DVE scalar_tensor_tensor scalar) -- the PE
runs a pure float32r matmul stream with no tn matmuls at all:

  - X_train split over 8 cores along N_train (12500 -> 100 x-tiles of
    128 points, zero-padded).  Per x-tile t the PE computes
    [128 points, 2048 queries] = x_t.T @ (2*X_test.T) as 4 matmuls of
    512 queries into 2 PSUM bank-pairs.
  - x-tiles fold (elementwise running max over tiles) into 4 group
    accumulators, g = t mod 4:
      groups 0-2: ScalarE stages PSUM->SBUF fp16 (ACTIVATE Identity
        with bias = -tn per partition), DVE folds fp16 at 2x;
      group 3: DVE scalar_tensor_tensor (ps + (-tn)) max acc, fp32,
        straight from PSUM.
  - Finals (once): fp16 accs convert to fp32, PE transposes each
    [128 points,128 queries] block (identity-permutation matmul) so
    queries land on partitions, ScalarE exits [128,512] slot arrays,
    one MAX8 + FIND_INDEX8 per 128-query chunk.
  - Slot p = 128*g + r expands to candidates {(g+4j)*128 + r, j<25};
    the host re-scores all 8*8*25 candidates per (core,query), merges
    exactly, majority-votes; margin cases are recomputed exactly.
"""
import os
import sys

sys.path.insert(0, "/opt/trn_rl_repo")

import numpy as np

N_CORES = 8
N_TRAIN = 100000
D = 128
N_TEST = 2048
K = 9
NUM_CLASSES = 3
SHARD = N_TRAIN // N_CORES          # 12500
XTILE = 128
N_XT = 100                          # x-tiles per core (padded)
SHARD_PAD = XTILE * N_XT            # 12800
N_GROUPS = 4
N_JT = N_XT // N_GROUPS             # 25 tiles per group
CHUNK = 128
N_CHUNKS = N_TEST // CHUNK          # 16
NSLOT = N_GROUPS * XTILE            # 512 slot space

_FW = N_TEST + SHARD_PAD

EPS_DEV = float(os.environ.get("KNN_EPS_DEV", 0.35))
PAD_SCORE = -1e30


def _build_program():
    import concourse.bacc as bacc
    import concourse.mybir as mybir
    from concourse.tile import TileContext

    AF = mybir.ActivationFunctionType
    MAX = mybir.AluOpType.max
    ADD = mybir.AluOpType.add

    nc = bacc.Bacc("TRN2", target_bir_lowering=False, debug=False)
    f_d = nc.dram_tensor("fblob", [128, _FW], mybir.dt.float32r,
                         kind="ExternalInput").ap()
    t_d = nc.dram_tensor("tnt", [128, N_XT], mybir.dt.float32,
                         kind="ExternalInput").ap()
    i_d = nc.dram_tensor("iden", [128, 128], mybir.dt.float32,
                         kind="ExternalInput").ap()
    o_d = nc.dram_tensor("out", [N_CHUNKS * CHUNK, 16], mybir.dt.float32,
                         kind="ExternalOutput").ap()

    with TileContext(nc) as tc:
        with tc.tile_pool(name="const", bufs=1) as cpool, \
             tc.tile_pool(name="stg", bufs=6) as spool, \
             tc.tile_pool(name="fin", bufs=4) as fpool, \
             tc.tile_pool(name="pp", bufs=2, space="PSUM") as pp:
            f_s = cpool.tile([128, _FW], mybir.dt.float32r)
            tnt = cpool.tile([128, N_XT], mybir.dt.float32)
            iden = cpool.tile([128, 128], mybir.dt.float32)
            nc.sync.dma_start(tnt[:], t_d[:])
            nc.sync.dma_start(iden[:], i_d[:])
            _cuts = [0, N_TEST + 2 * XTILE, N_TEST + 25 * XTILE,
                     N_TEST + 50 * XTILE, N_TEST + 75 * XTILE, _FW]
            for a, b in zip(_cuts[:-1], _cuts[1:]):
                nc.sync.dma_start(f_s[:, a:b], f_d[:, a:b])
            q_s = f_s[:, 0:N_TEST]          # [128, 2048] (2*X_test).T f32r
            x_s = f_s[:, N_TEST:_FW]        # [128, 12800] shard.T f32r

            accH = [cpool.tile([128, 2048], mybir.dt.float16,
                               tag=f"accH{g}", name=f"accH{g}")
                    for g in range(3)]
            accA = cpool.tile([128, 2048], mybir.dt.float32, tag="accA")
            accF = [cpool.tile([128, 2048], mybir.dt.float32,
                               tag=f"accF{g}", name=f"accF{g}")
                    for g in range(3)]
            nc.gpsimd.memset(accA[:], -3e38)

            for t in range(N_XT):
                g = t % N_GROUPS
                xt = x_s[:, t * XTILE:(t + 1) * XTILE]
                bias = tnt[:, t:t + 1]
                ps = pp.tile([128, 2048], mybir.dt.float32, tag="pp")
                # matmul output is capped at one PSUM bank (512 fp32)
                for h in range(4):
                    nc.tensor.matmul(ps[:, h * 512:(h + 1) * 512], xt,
                                     q_s[:, h * 512:(h + 1) * 512],
                                     start=True, stop=True,
                                     skip_group_check=True)
                if g == 3:
                    nc.vector.scalar_tensor_tensor(
                        accA[:], ps[:], bias, accA[:], op0=ADD, op1=MAX)
                elif t == g:
                    nc.scalar.activation(accH[g][:], ps[:], AF.Identity,
                                         bias=bias)
                else:
                    tmp = spool.tile([128, 2048], mybir.dt.float16,
                                     tag="tmp")
                    nc.scalar.activation(tmp[:], ps[:], AF.Identity,
                                         bias=bias)
                    nc.vector.tensor_tensor(accH[g][:], accH[g][:], tmp[:],
                                            op=MAX)
            # finals: results accumulate into one collect tile, single DMA
            coll = cpool.tile([128, 16 * N_CHUNKS], mybir.dt.float32)
            for g in range(3):
                nc.scalar.activation(accF[g][:], accH[g][:], AF.Identity)
            srcs = accF + [accA]
            for c in range(N_CHUNKS):
                psT = pp.tile([128, 2048], mybir.dt.float32, tag="pp")
                for g in range(N_GROUPS):
                    nc.tensor.matmul(
                        psT[:, g * 128:(g + 1) * 128],
                        srcs[g][:, c * CHUNK:(c + 1) * CHUNK], iden[:],
                        start=True, stop=True, is_transpose=True,
                        skip_group_check=True)
                o = c * 16
                fin = fpool.tile([128, NSLOT], mybir.dt.float32, tag="fin")
                nc.scalar.copy(fin[:], psT[:, 0:NSLOT])
                nc.vector.max(coll[:, o:o + 8], fin[:])
                nc.vector.max_index(
                    coll[:, o + 8:o + 16].bitcast(mybir.dt.uint32),
                    coll[:, o:o + 8], fin[:])
            # per-chunk output DMAs from the collect tile (coll is never
            # recycled, so these no longer stall the finals pipeline)
            for c in range(N_CHUNKS):
                nc.sync.dma_start(o_d[c * CHUNK:(c + 1) * CHUNK, :],
                                  coll[:, c * 16:(c + 1) * 16])
    nc.compile()
    return nc


def _prep_inputs(X_train, X_test):
    XT = np.ascontiguousarray(X_train.T.astype(np.float32))        # [128,100000]
    q2t = np.ascontiguousarray((2.0 * X_test.astype(np.float32)).T)
    tn64 = (X_train.astype(np.float64) ** 2).sum(1)                # [100000]
    iden = np.eye(128, dtype=np.float32)
    in_maps = []
    for c in range(N_CORES):
        sl = slice(c * SHARD, (c + 1) * SHARD)
        xpad = np.zeros((128, SHARD_PAD), np.float32)
        xpad[:, 0:SHARD] = XT[:, sl]
        tnpad = np.full(SHARD_PAD, PAD_SCORE, np.float64)
        tnpad[0:SHARD] = -tn64[sl]
        fblob = np.ascontiguousarray(np.concatenate([q2t, xpad], axis=1))
        tnt = np.ascontiguousarray(
            tnpad.reshape(N_XT, XTILE).T.astype(np.float32))       # [128,100]
        in_maps.append({"fblob": fblob, "tnt": tnt, "iden": iden})
    return in_maps, tn64


def _reference_style_batch(qs32, X_train32, tn32, qn32):
    """Reference-fp32 top-K for a batch of queries -> [F, K] indices."""
    F = qs32.shape[0]
    out = np.empty((F, K), np.int64)
    B = 64
    for a in range(0, F, B):
        b = min(a + B, F)
        d2 = qn32[a:b][:, None] + tn32[None, :] - 2.0 * (qs32[a:b] @ X_train32.T)
        part = np.argpartition(d2, K + 8, axis=1)[:, :K + 8]
        pv = np.take_along_axis(d2, part, axis=1)
        ordr = np.lexsort((part, pv), axis=1)[:, :K]
        out[a:b] = np.take_along_axis(part, ordr, axis=1)
    return out


def _host_merge(X_train, y_train, X_test, vals, idxs, tn64, diag=None):
    """vals/idxs: [n_cores, nq, 8] device folded top-8 (values, slot).

    Slot p = 128*g + r expands to {(g + 4*j)*128 + r : j in 0..24}.
    """
    nq = vals.shape[1]
    n_cores = vals.shape[0]

    slots = idxs.astype(np.int64)                                  # [C,nq,8]
    bad = (slots >= NSLOT).any(axis=(0, 2))
    ss_ = np.sort(slots, axis=2)
    dup = (np.diff(ss_, axis=2) == 0).any(axis=(0, 2))
    slots = np.clip(slots, 0, NSLOT - 1)

    gg = slots // XTILE                                            # [C,nq,8]
    rr = slots % XTILE
    jj = np.arange(N_JT, dtype=np.int64)                           # [25]
    loc = (gg[..., None] + N_GROUPS * jj) * XTILE + rr[..., None]  # [C,nq,8,25]
    valid = loc < SHARD
    gidx = np.where(valid, loc, 0) + (np.arange(n_cores, dtype=np.int64)
                                      [:, None, None, None] * SHARD)
    n_exp = 8 * N_JT                                               # 200
    gidx = gidx.reshape(n_cores, nq, n_exp).transpose(1, 0, 2)
    valid = valid.reshape(n_cores, nq, n_exp).transpose(1, 0, 2)
    gidx_f = gidx.reshape(nq, n_cores * n_exp)
    valid_f = valid.reshape(nq, n_cores * n_exp)

    X32 = X_train.astype(np.float32)
    tn32 = (X32.astype(np.float64) ** 2).sum(1).astype(np.float32)
    q32 = X_test.astype(np.float32)
    s32 = np.empty((nq, n_cores * n_exp), dtype=np.float32)
    QB = 256
    for a in range(0, nq, QB):
        b = min(a + QB, nq)
        Xc = X32[gidx_f[a:b]]
        s32[a:b] = (2.0 * np.matmul(Xc, q32[a:b, :, None])[..., 0]
                    - tn32[gidx_f[a:b]])
    s32[~valid_f] = PAD_SCORE

    s_core = s32.reshape(nq, n_cores, n_exp)
    e8 = -np.partition(-s_core, 7, axis=2)[:, :, 7]

    slot_max = s_core.reshape(nq, n_cores, 8, N_JT).max(3)
    dev_err = np.abs(vals.transpose(1, 0, 2).astype(np.float32) - slot_max)
    if diag is not None:
        diag["max_dev_err"] = float(dev_err.max())
        diag["p99_dev_err"] = float(np.quantile(dev_err, 0.99))
    flag_eps = (dev_err > 0.5 * EPS_DEV).any(axis=(1, 2))

    TOP = 64
    top_i = np.argpartition(-s32, TOP - 1, axis=1)[:, :TOP]
    cand = np.take_along_axis(gidx_f, top_i, axis=1)
    Xc64 = X_train[cand].astype(np.float64)
    s64 = (2.0 * np.matmul(Xc64,
                           X_test.astype(np.float64)[:, :, None])[..., 0]
           - tn64[cand])
    cand_valid = np.take_along_axis(valid_f, top_i, axis=1)
    s64[~cand_valid] = PAD_SCORE
    order = np.argsort(-s64, axis=1, kind="stable")
    s_sorted = np.take_along_axis(s64, order, axis=1)
    top9 = np.take_along_axis(cand, order[:, :K], axis=1)
    v9 = s_sorted[:, K - 1]
    v10 = s_sorted[:, K]

    flag_hidden = (e8.astype(np.float64) + EPS_DEV >= v9[:, None]).any(1)
    top9_core = top9 // SHARD
    counts_core = np.zeros((nq, n_cores), dtype=np.int64)
    for c in range(n_cores):
        counts_core[:, c] = (top9_core == c).sum(1)
    flag_dom = (counts_core >= 8).any(1)
    flag_tie = (v9 - v10) < 1e-4
    t9s = np.sort(top9, axis=1)
    dup9 = (np.diff(t9s, axis=1) == 0).any(1)

    flagged = np.where(bad | dup | flag_hidden | flag_dom | flag_tie
                       | flag_eps | dup9)[0]
    if len(flagged):
        qn32 = (q32 * q32).sum(1)
        top9[flagged] = _reference_style_batch(q32[flagged], X32, tn32,
                                               qn32[flagged])

    labels = y_train[top9]
    counts = (labels[:, :, None] ==
              np.arange(NUM_CLASSES, dtype=labels.dtype)[None, None, :]).sum(1)
    preds = counts.argmax(1).astype(np.int32)
    return preds, len(flagged)


_cached = {}


def run_device(X_train, X_test, trace=False):
    from concourse.bass_utils import run_bass_kernel_spmd

    in_maps, tn64 = _prep_inputs(X_train, X_test)
    if "nc" not in _cached:
        _cached["nc"] = _build_program()
    nc = _cached["nc"]
    res = run_bass_kernel_spmd(nc, in_maps, core_ids=list(range(N_CORES)),
                               trace=trace)
    vals = np.stack([res.results[c]["out"][:, 0:8] for c in range(N_CORES)])
    idxs = np.stack([res.results[c]["out"][:, 8:16].view(np.uint32)
                     for c in range(N_CORES)])
    return vals, idxs, tn64, res


def kernel(X_train, y_train, X_test):
    X_train = np.asarray(X_train)
    y_train = np.asarray(y_train)
    X_test = np.asarray(X_test)
    vals, idxs, tn64, _ = run_device(X_train, X_test, trace=False)
    nq = vals.shape[1]
    preds, _n_flagged = _host_merge(X_train, y_train, X_test[:nq], vals, idxs,
                                    tn64)
    return preds
